# revision 9
# baseline (speedup 1.0000x reference)
"""GCN (2-layer, symmetric-normalized, self-loops) on 8 Trainium2 NeuronCores.

Strategy (graph/data parallel):
  - Nodes are sorted by degree, grouped into 128-node chunks, and chunks are
    dealt round-robin to the 8 cores (load balance + tight ELL padding).
  - Normalization is factored: out_i = dinv_i * sum_{j in N(i)+self} (dinv_j h_j)
    so each layer is: GEMM -> row-scale by dinv -> AllGather the bf16 table ->
    per-dst-chunk padded gather (dma_gather, 512B rows) -> tree-reduce (DVE) ->
    epilogue.  Layer 2 ends with a small GEMM by W2 (PE transpose first).
  - All per-edge work is on-device; the host only builds int16 index tables.
"""

import os
import numpy as np
import ml_dtypes

N, F_IN, H, C, E = 20000, 512, 256, 40, 640000
TEMPERATURE = 1.0
NCORES = 8
P = 128
M = 20                     # node chunks per core
NS = M * P                 # node slots per core (2560)
NT = NCORES * NS           # total slots (20480)

_BF16 = ml_dtypes.bfloat16
_prog_cache = {}
last_result = None         # BassKernelResults of the most recent run (for test.py)


def _install_trace_shim():
    """Provide antenv.axon_hooks (NTFF profiling) missing from this image."""
    import contextlib, ctypes, sys, types
    import antenv

    if "antenv.axon_hooks" in sys.modules:
        return

    so_path = "/opt/axon/libaxon_pjrt.so"

    def _build_hook():
        try:
            lib = ctypes.CDLL(so_path)
        except OSError:
            return None
        if not hasattr(lib, "axon_start_nrt_profile"):
            return None
        lib.axon_start_nrt_profile.argtypes = [
            ctypes.POINTER(ctypes.c_int64),
            ctypes.c_size_t,
        ]
        lib.axon_start_nrt_profile.restype = ctypes.c_int64
        lib.axon_stop_nrt_profile.argtypes = [ctypes.c_char_p]
        lib.axon_stop_nrt_profile.restype = ctypes.c_int64

        @contextlib.contextmanager
        def _hook(output_dir, device_ids):
            import jax

            jax.devices()
            if device_ids:
                ids = (ctypes.c_int64 * len(device_ids))(*device_ids)
                rc = lib.axon_start_nrt_profile(ids, len(device_ids))
            else:
                rc = lib.axon_start_nrt_profile(None, 0)
            if rc != 0:
                raise RuntimeError(f"axon_start_nrt_profile rc={rc}")
            try:
                yield
            finally:
                lib.axon_stop_nrt_profile(output_dir.encode())

        return _hook

    mod = types.ModuleType("antenv.axon_hooks")
    _state = {"hook": None, "built": False}

    def get_axon_ntff_profile_hook():
        if not _state["built"]:
            _state["hook"] = _build_hook()
            _state["built"] = True
        return _state["hook"]

    def set_axon_ntff_profile_hook(h):
        _state["hook"] = h
        _state["built"] = True

    mod.get_axon_ntff_profile_hook = get_axon_ntff_profile_hook
    mod.set_axon_ntff_profile_hook = set_axon_ntff_profile_hook
    antenv.axon_hooks = mod
    sys.modules["antenv.axon_hooks"] = mod

    # hlo_convert binary is absent in this image; disable HLO annotation.
    import gauge.trn_perfetto as tp

    if not getattr(tp.main, "_no_hlo_patch", False):
        orig = tp.main

        def main(*a, **k):
            k["annotate_hlo"] = False
            return orig(*a, **k)

        main._no_hlo_patch = True
        tp.main = main


def _tree_reduce(nc, gt, D):
    """In-place sum of gt[:, 0:D, :] into gt[:, 0, :] (bf16 pairwise tree)."""
    n = D
    while n > 1:
        h = n // 2
        nc.vector.tensor_add(gt[:, 0:h, :], gt[:, 0:h, :], gt[:, h : 2 * h, :])
        if n % 2 == 1:
            nc.vector.tensor_add(gt[:, 0, :], gt[:, 0, :], gt[:, n - 1, :])
        n = h


def _build_program(Dlist):
    from contextlib import ExitStack

    import concourse.bacc as bacc
    import concourse.tile as tile
    import concourse.mybir as mybir
    from concourse.library_config import mlp

    bf16 = mybir.dt.bfloat16
    f32 = mybir.dt.float32
    i16 = mybir.dt.int16
    SIDX = 8 * sum(Dlist)

    phase_limit = int(os.environ.get("GCN_PHASE_LIMIT", "99"))
    nc = bacc.Bacc("TRN2", num_devices=NCORES, target_bir_lowering=True)

    xT_d = nc.dram_tensor("xT", [F_IN, NS], bf16, kind="ExternalInput")
    w1_d = nc.dram_tensor("w1", [F_IN, H], bf16, kind="ExternalInput")
    w2_d = nc.dram_tensor("w2", [H, C], bf16, kind="ExternalInput")
    b1_d = nc.dram_tensor("b1bc", [P, H], f32, kind="ExternalInput")
    b2_d = nc.dram_tensor("b2bc", [P, C], f32, kind="ExternalInput")
    dinv_d = nc.dram_tensor("dinv", [P, M], f32, kind="ExternalInput")
    dinv2_d = nc.dram_tensor("dinv2", [P, M], f32, kind="ExternalInput")
    idx_d = nc.dram_tensor("idx", [P, SIDX], i16, kind="ExternalInput")
    id_d = nc.dram_tensor("ident", [P, P], bf16, kind="ExternalInput")
    out_d = nc.dram_tensor("out", [P, M * C], f32, kind="ExternalOutput")

    with tile.TileContext(nc) as tc, ExitStack() as ctx:
        dp = ctx.enter_context(tc.tile_pool(name="dram", bufs=1, space="DRAM"))
        cp = ctx.enter_context(tc.tile_pool(name="const", bufs=1))
        pp1 = ctx.enter_context(tc.tile_pool(name="ps1", bufs=2, space="PSUM"))
        ppt = ctx.enter_context(tc.tile_pool(name="pt", bufs=3, space="PSUM"))
        pp2 = ctx.enter_context(tc.tile_pool(name="ps2", bufs=2, space="PSUM"))
        hp = ctx.enter_context(tc.tile_pool(name="hp", bufs=3))
        gp = ctx.enter_context(tc.tile_pool(name="gp", bufs=2))
        sp = ctx.enter_context(tc.tile_pool(name="sp", bufs=3))

        nc.gpsimd.load_library(mlp)

        ag1_in = dp.tile([NS, H], bf16)
        ag1_out = dp.tile([NT, H], bf16, addr_space="Shared")
        ag2_in = dp.tile([NS, H], bf16)
        ag2_out = dp.tile([NT, H], bf16, addr_space="Shared")

        w1sb = []
        for k in range(4):
            t = cp.tile([P, H], bf16, name=f"w1_{k}")
            nc.sync.dma_start(t[:], w1_d[k * P : (k + 1) * P, :])
            w1sb.append(t)
        w2sb = []
        for k in range(2):
            t = cp.tile([P, C], bf16, name=f"w2_{k}")
            nc.sync.dma_start(t[:], w2_d[k * P : (k + 1) * P, :])
            w2sb.append(t)
        xts = []
        for k in range(4):
            t = cp.tile([P, NS], bf16, name=f"xt_{k}")
            nc.sync.dma_start(t[:], xT_d[k * P : (k + 1) * P, :])
            xts.append(t)
        b1bc = cp.tile([P, H], f32, name="b1bc")
        nc.sync.dma_start(b1bc[:], b1_d[:])
        b2bc = cp.tile([P, C], f32, name="b2bc")
        nc.sync.dma_start(b2bc[:], b2_d[:])
        dinvs = cp.tile([P, M], f32, name="dinvs")
        nc.sync.dma_start(dinvs[:], dinv_d[:])
        dinv2s = cp.tile([P, M], f32, name="dinv2s")
        nc.sync.dma_start(dinv2s[:], dinv2_d[:])
        idxsb = cp.tile([P, SIDX], i16, name="idxsb")
        nc.sync.dma_start(idxsb[:], idx_d[:])
        identsb = cp.tile([P, P], bf16, name="identsb")
        nc.sync.dma_start(identsb[:], id_d[:])
        outsb = cp.tile([P, M * C], f32, name="outsb")
        if phase_limit < 5:
            nc.vector.memset(outsb[:], 0.0)

        # ---- layer-1 GEMM: h1 = dinv * (x @ W1), per 128-node chunk ----
        for m in range(M):
            ps = pp1.tile([P, H], f32, tag="ps1")
            for k in range(4):
                nc.tensor.matmul(
                    ps[:],
                    xts[k][:, m * P : (m + 1) * P],
                    w1sb[k][:],
                    start=(k == 0),
                    stop=(k == 3),
                )
            h1 = hp.tile([P, H], bf16, tag="h1")
            nc.scalar.mul(h1[:], ps[:], dinvs[:, m : m + 1])
            nc.sync.dma_start(ag1_in[m * P : (m + 1) * P, :], h1[:])

        if phase_limit >= 2:
            nc.gpsimd.collective_compute(
                "AllGather",
                mybir.AluOpType.bypass,
                replica_groups=[list(range(NCORES))],
                ins=[ag1_in[:].opt()],
                outs=[ag1_out[:].opt()],
            )

        # ---- layer-1 aggregation + relu -> h2 table ----
        ph3_mode = int(os.environ.get("GCN_PH3_MODE", "3"))
        off = 0
        for m in range(M if phase_limit >= 3 else 0):
            D = Dlist[m]
            gt = gp.tile([P, D, H], bf16, tag="gt")
            nc.gpsimd.dma_gather(
                gt[:], ag1_out[:], idxsb[:, off : off + 8 * D], P * D, P * D, H,
                single_packet=False,
            )
            if ph3_mode >= 2:
                _tree_reduce(nc, gt, D)
            if ph3_mode >= 3:
                b1dt = sp.tile([P, H], f32, tag="b1d")
                nc.vector.tensor_scalar_mul(b1dt[:], b1bc[:], dinvs[:, m : m + 1])
                e1 = sp.tile([P, H], f32, tag="e1")
                nc.vector.scalar_tensor_tensor(
                    e1[:],
                    gt[:, 0, :],
                    dinv2s[:, m : m + 1],
                    b1dt[:],
                    mybir.AluOpType.mult,
                    mybir.AluOpType.add,
                )
                h2 = hp.tile([P, H], bf16, tag="h2")
                nc.scalar.activation(h2[:], e1[:], mybir.ActivationFunctionType.Relu)
                nc.sync.dma_start(ag2_in[m * P : (m + 1) * P, :], h2[:])
            else:
                h2 = hp.tile([P, H], bf16, tag="h2")
                nc.vector.tensor_copy(h2[:], gt[:, 0, :])
                nc.sync.dma_start(ag2_in[m * P : (m + 1) * P, :], h2[:])
            off += 8 * D

        if phase_limit >= 4:
            nc.gpsimd.collective_compute(
                "AllGather",
                mybir.AluOpType.bypass,
                replica_groups=[list(range(NCORES))],
                ins=[ag2_in[:].opt()],
                outs=[ag2_out[:].opt()],
            )

        # ---- layer-2 aggregation + GEMM by W2 ----
        off = 0
        for m in range(M if phase_limit >= 5 else 0):
            D = Dlist[m]
            gt = gp.tile([P, D, H], bf16, tag="gt")
            nc.gpsimd.dma_gather(
                gt[:], ag2_out[:], idxsb[:, off : off + 8 * D], P * D, P * D, H,
                single_packet=False,
            )
            _tree_reduce(nc, gt, D)
            s2d = hp.tile([P, H], bf16, tag="s2d")
            nc.vector.tensor_scalar_mul(s2d[:], gt[:, 0, :], dinvs[:, m : m + 1])
            sts = []
            for kk in range(2):
                pt = ppt.tile([P, P], bf16, tag="pt")
                nc.tensor.transpose(pt[:], s2d[:, kk * P : (kk + 1) * P], identsb[:])
                st = sp.tile([P, P], bf16, tag="st")
                nc.scalar.copy(st[:], pt[:])
                sts.append(st)
            ps2 = pp2.tile([P, C], f32, tag="ps2")
            for kk in range(2):
                nc.tensor.matmul(
                    ps2[:], sts[kk][:], w2sb[kk][:], start=(kk == 0), stop=(kk == 1)
                )
            nc.vector.tensor_add(outsb[:, m * C : (m + 1) * C], ps2[:], b2bc[:])
            off += 8 * D

        nc.sync.dma_start(out_d[:], outsb[:])

    nc.compile()
    return nc


def kernel(x, edge_index, W1, b1, W2, b2):
    global last_result
    x = np.asarray(x, dtype=np.float32)
    edge_index = np.asarray(edge_index)
    W1 = np.asarray(W1, dtype=np.float32)
    b1 = np.asarray(b1, dtype=np.float32)
    W2 = np.asarray(W2, dtype=np.float32)
    b2 = np.asarray(b2, dtype=np.float32)

    n = x.shape[0]
    src = edge_index[0].astype(np.int64)
    dst = edge_index[1].astype(np.int64)

    # ---- normalization ----
    deg_in = np.bincount(dst, minlength=n).astype(np.int64)
    degv = deg_in.astype(np.float64) + 1.0
    dinv = (1.0 / np.sqrt(degv)).astype(np.float32)

    # ---- degree-sorted slot assignment ----
    order = np.argsort(-degv, kind="stable")          # rank -> node
    ranks = np.arange(NT, dtype=np.int64)
    g = ranks // P
    slot_of_rank = (g % NCORES) * NS + (g // NCORES) * P + (ranks % P)
    node_of_slot = np.full(NT, -1, dtype=np.int64)
    node_of_slot[slot_of_rank[:n]] = order
    slot_of_node = np.empty(n, dtype=np.int64)
    slot_of_node[order] = slot_of_rank[:n]

    dslot = slot_of_node[dst]
    sslot = slot_of_node[src]

    counts = np.bincount(dslot, minlength=NT).astype(np.int64)
    cnt1 = counts + 1                                  # + self edge
    mpos_of_slot = (np.arange(NT) % NS) // P
    Dm = np.zeros(M, dtype=np.int64)
    np.maximum.at(Dm, mpos_of_slot, cnt1)
    Dlist = tuple(int(v) for v in Dm)
    Dmax = int(Dm.max())

    PAD_SLOT = NT - 1 if n < NT else NT - 1            # guaranteed dummy (zero row)
    A = np.full((NT, Dmax), PAD_SLOT, dtype=np.int64)
    A[:, 0] = np.arange(NT)                            # self edge
    eorder = np.argsort(dslot, kind="stable")
    ds = dslot[eorder]
    ss = sslot[eorder]
    starts = np.zeros(NT + 1, dtype=np.int64)
    np.cumsum(counts, out=starts[1:])
    pos = np.arange(E, dtype=np.int64) - starts[ds]
    A[ds, pos + 1] = ss

    # ---- per-core inputs ----
    x_bf = x.astype(_BF16)
    w1_bf = W1.astype(_BF16)
    w2_bf = W2.astype(_BF16)
    b1bc = np.broadcast_to(b1, (P, H)).astype(np.float32).copy()
    b2bc = np.broadcast_to(b2, (P, C)).astype(np.float32).copy()
    ident = np.eye(P, dtype=np.float32).astype(_BF16)

    dinv_slots = np.zeros(NT, dtype=np.float32)
    real = node_of_slot >= 0
    dinv_slots[real] = dinv[node_of_slot[real]]

    in_maps = []
    for c in range(NCORES):
        slots = np.arange(c * NS, (c + 1) * NS)
        nos = node_of_slot[slots]
        xs = np.zeros((NS, F_IN), dtype=_BF16)
        r = nos >= 0
        xs[r] = x_bf[nos[r]]
        dv = dinv_slots[slots].reshape(M, P).T.copy()   # [128, M]
        blocks = []
        for m in range(M):
            blk = A[slots[m * P : (m + 1) * P], : Dlist[m]]   # [128, D]
            flat = blk.T.reshape(-1)                          # i = j*128 + p
            blocks.append(flat.reshape(-1, 16).T)             # [16, 8*D]
        idx16 = np.concatenate(blocks, axis=1)
        idx128 = np.tile(idx16, (8, 1)).astype(np.int16)
        in_maps.append(
            {
                "xT": np.ascontiguousarray(xs.T),
                "w1": w1_bf,
                "w2": w2_bf,
                "b1bc": b1bc,
                "b2bc": b2bc,
                "dinv": dv,
                "dinv2": dv * dv,
                "idx": idx128,
                "ident": ident,
            }
        )

    # ---- build + run ----
    from concourse.bass_utils import run_bass_kernel_spmd

    trace = bool(int(os.environ.get("BASS_GCN_TRACE", "0")))
    if trace:
        _install_trace_shim()

    key = Dlist
    if key not in _prog_cache:
        _prog_cache[key] = _build_program(Dlist)
    nc = _prog_cache[key]

    res = run_bass_kernel_spmd(nc, in_maps, list(range(NCORES)), trace=trace)
    last_result = res

    # ---- gather + unpermute ----
    out_full = np.empty((n, C), dtype=np.float32)
    for c in range(NCORES):
        oc = res.results[c]["out"].reshape(P, M, C).transpose(1, 0, 2).reshape(NS, C)
        slots = np.arange(c * NS, (c + 1) * NS)
        nos = node_of_slot[slots]
        r = nos >= 0
        out_full[nos[r]] = oc[r]

    return (out_full / np.float32(TEMPERATURE)).astype(np.float32)


# revision 10
# speedup vs baseline: 1.5995x; 1.5995x over previous
"""GCN (2-layer, symmetric-normalized, self-loops) on 8 Trainium2 NeuronCores.

Strategy (graph/data parallel):
  - Nodes are sorted by degree, grouped into 128-node chunks, and chunks are
    dealt round-robin to the 8 cores (load balance + tight ELL padding).
  - Normalization is factored: out_i = dinv_i * sum_{j in N(i)+self} (dinv_j h_j)
    so each layer is: GEMM -> row-scale by dinv -> AllGather the bf16 table ->
    per-dst-chunk padded gather (dma_gather, 512B rows) -> tree-reduce (DVE) ->
    epilogue.  Layer 2 ends with a small GEMM by W2 (PE transpose first).
  - All per-edge work is on-device; the host only builds int16 index tables.
"""

import os
import numpy as np
import ml_dtypes

N, F_IN, H, C, E = 20000, 512, 256, 40, 640000
TEMPERATURE = 1.0
NCORES = 8
P = 128
M = 20                     # node chunks per core
NS = M * P                 # node slots per core (2560)
NT = NCORES * NS           # total slots (20480)

_BF16 = ml_dtypes.bfloat16
_prog_cache = {}
last_result = None         # BassKernelResults of the most recent run (for test.py)


def _install_trace_shim():
    """Provide antenv.axon_hooks (NTFF profiling) missing from this image."""
    import contextlib, ctypes, sys, types
    import antenv

    if "antenv.axon_hooks" in sys.modules:
        return

    so_path = "/opt/axon/libaxon_pjrt.so"

    def _build_hook():
        try:
            lib = ctypes.CDLL(so_path)
        except OSError:
            return None
        if not hasattr(lib, "axon_start_nrt_profile"):
            return None
        lib.axon_start_nrt_profile.argtypes = [
            ctypes.POINTER(ctypes.c_int64),
            ctypes.c_size_t,
        ]
        lib.axon_start_nrt_profile.restype = ctypes.c_int64
        lib.axon_stop_nrt_profile.argtypes = [ctypes.c_char_p]
        lib.axon_stop_nrt_profile.restype = ctypes.c_int64

        @contextlib.contextmanager
        def _hook(output_dir, device_ids):
            import jax

            jax.devices()
            if device_ids:
                ids = (ctypes.c_int64 * len(device_ids))(*device_ids)
                rc = lib.axon_start_nrt_profile(ids, len(device_ids))
            else:
                rc = lib.axon_start_nrt_profile(None, 0)
            if rc != 0:
                raise RuntimeError(f"axon_start_nrt_profile rc={rc}")
            try:
                yield
            finally:
                lib.axon_stop_nrt_profile(output_dir.encode())

        return _hook

    mod = types.ModuleType("antenv.axon_hooks")
    _state = {"hook": None, "built": False}

    def get_axon_ntff_profile_hook():
        if not _state["built"]:
            _state["hook"] = _build_hook()
            _state["built"] = True
        return _state["hook"]

    def set_axon_ntff_profile_hook(h):
        _state["hook"] = h
        _state["built"] = True

    mod.get_axon_ntff_profile_hook = get_axon_ntff_profile_hook
    mod.set_axon_ntff_profile_hook = set_axon_ntff_profile_hook
    antenv.axon_hooks = mod
    sys.modules["antenv.axon_hooks"] = mod

    # hlo_convert binary is absent in this image; disable HLO annotation.
    import gauge.trn_perfetto as tp

    if not getattr(tp.main, "_no_hlo_patch", False):
        orig = tp.main

        def main(*a, **k):
            k["annotate_hlo"] = False
            return orig(*a, **k)

        main._no_hlo_patch = True
        tp.main = main


def _tree_reduce(nc, gt, D):
    """In-place sum of gt[:, 0:D, :] into gt[:, 0, :] (bf16 pairwise tree)."""
    n = D
    while n > 1:
        h = n // 2
        nc.vector.tensor_add(gt[:, 0:h, :], gt[:, 0:h, :], gt[:, h : 2 * h, :])
        if n % 2 == 1:
            nc.vector.tensor_add(gt[:, 0, :], gt[:, 0, :], gt[:, n - 1, :])
        n = h


def _build_program(Dlist):
    from contextlib import ExitStack

    import concourse.bacc as bacc
    import concourse.tile as tile
    import concourse.mybir as mybir
    from concourse.library_config import mlp

    bf16 = mybir.dt.bfloat16
    f32 = mybir.dt.float32
    i16 = mybir.dt.int16
    SIDX = 8 * sum(Dlist)

    phase_limit = int(os.environ.get("GCN_PHASE_LIMIT", "99"))
    nc = bacc.Bacc("TRN2", num_devices=NCORES, target_bir_lowering=True, num_swdge_queues=4)

    xT_d = nc.dram_tensor("xT", [F_IN, NS], bf16, kind="ExternalInput")
    w1_d = nc.dram_tensor("w1", [F_IN, H], bf16, kind="ExternalInput")
    w2_d = nc.dram_tensor("w2", [H, C], bf16, kind="ExternalInput")
    b1_d = nc.dram_tensor("b1bc", [P, H], f32, kind="ExternalInput")
    b2_d = nc.dram_tensor("b2bc", [P, C], f32, kind="ExternalInput")
    dinv_d = nc.dram_tensor("dinv", [P, M], f32, kind="ExternalInput")
    dinv2_d = nc.dram_tensor("dinv2", [P, M], f32, kind="ExternalInput")
    idx_d = nc.dram_tensor("idx", [P, SIDX], i16, kind="ExternalInput")
    id_d = nc.dram_tensor("ident", [P, P], bf16, kind="ExternalInput")
    out_d = nc.dram_tensor("out", [P, M * C], f32, kind="ExternalOutput")

    with tile.TileContext(nc) as tc, ExitStack() as ctx:
        dp = ctx.enter_context(tc.tile_pool(name="dram", bufs=1, space="DRAM"))
        cp = ctx.enter_context(tc.tile_pool(name="const", bufs=1))
        pp1 = ctx.enter_context(tc.tile_pool(name="ps1", bufs=2, space="PSUM"))
        ppt = ctx.enter_context(tc.tile_pool(name="pt", bufs=3, space="PSUM"))
        pp2 = ctx.enter_context(tc.tile_pool(name="ps2", bufs=2, space="PSUM"))
        hp = ctx.enter_context(tc.tile_pool(name="hp", bufs=3))
        gp = ctx.enter_context(tc.tile_pool(name="gp", bufs=2))
        sp = ctx.enter_context(tc.tile_pool(name="sp", bufs=3))

        nc.gpsimd.load_library(mlp)

        ag1_in = dp.tile([NS, H], bf16)
        ag1_out = dp.tile([NT, H], bf16, addr_space="Shared")
        ag2_in = dp.tile([NS, H], bf16)
        ag2_out = dp.tile([NT, H], bf16, addr_space="Shared")

        w1sb = []
        for k in range(4):
            t = cp.tile([P, H], bf16, name=f"w1_{k}")
            nc.sync.dma_start(t[:], w1_d[k * P : (k + 1) * P, :])
            w1sb.append(t)
        w2sb = []
        for k in range(2):
            t = cp.tile([P, C], bf16, name=f"w2_{k}")
            nc.sync.dma_start(t[:], w2_d[k * P : (k + 1) * P, :])
            w2sb.append(t)
        xts = []
        for k in range(4):
            t = cp.tile([P, NS], bf16, name=f"xt_{k}")
            nc.sync.dma_start(t[:], xT_d[k * P : (k + 1) * P, :])
            xts.append(t)
        b1bc = cp.tile([P, H], f32, name="b1bc")
        nc.sync.dma_start(b1bc[:], b1_d[:])
        b2bc = cp.tile([P, C], f32, name="b2bc")
        nc.sync.dma_start(b2bc[:], b2_d[:])
        dinvs = cp.tile([P, M], f32, name="dinvs")
        nc.sync.dma_start(dinvs[:], dinv_d[:])
        dinv2s = cp.tile([P, M], f32, name="dinv2s")
        nc.sync.dma_start(dinv2s[:], dinv2_d[:])
        idxsb = cp.tile([P, SIDX], i16, name="idxsb")
        nc.sync.dma_start(idxsb[:], idx_d[:])
        identsb = cp.tile([P, P], bf16, name="identsb")
        nc.sync.dma_start(identsb[:], id_d[:])
        outsb = cp.tile([P, M * C], f32, name="outsb")
        if phase_limit < 5:
            nc.vector.memset(outsb[:], 0.0)

        # ---- layer-1 GEMM: h1 = dinv * (x @ W1), per 128-node chunk ----
        for m in range(M):
            ps = pp1.tile([P, H], f32, tag="ps1")
            for k in range(4):
                nc.tensor.matmul(
                    ps[:],
                    xts[k][:, m * P : (m + 1) * P],
                    w1sb[k][:],
                    start=(k == 0),
                    stop=(k == 3),
                )
            h1 = hp.tile([P, H], bf16, tag="h1")
            nc.scalar.mul(h1[:], ps[:], dinvs[:, m : m + 1])
            nc.sync.dma_start(ag1_in[m * P : (m + 1) * P, :], h1[:])

        if phase_limit >= 2:
            nc.gpsimd.collective_compute(
                "AllGather",
                mybir.AluOpType.bypass,
                replica_groups=[list(range(NCORES))],
                ins=[ag1_in[:].opt()],
                outs=[ag1_out[:].opt()],
            )

        # ---- layer-1 aggregation + relu -> h2 table ----
        ph3_mode = int(os.environ.get("GCN_PH3_MODE", "3"))
        off = 0
        for m in range(M if phase_limit >= 3 else 0):
            D = Dlist[m]
            gt = gp.tile([P, D, H], bf16, tag="gt")
            for j0 in range(0, D, 16):
                j1 = min(j0 + 16, D)
                nc.gpsimd.dma_gather(
                    gt[:, j0:j1, :],
                    ag1_out[:],
                    idxsb[:, off + 8 * j0 : off + 8 * j1],
                    P * (j1 - j0),
                    P * (j1 - j0),
                    H,
                    single_packet=False,
                    queue_num=(m * 4 + j0 // 16) % 4,
                )
            if ph3_mode >= 2:
                _tree_reduce(nc, gt, D)
            if ph3_mode >= 3:
                b1dt = sp.tile([P, H], f32, tag="b1d")
                nc.vector.tensor_scalar_mul(b1dt[:], b1bc[:], dinvs[:, m : m + 1])
                e1 = sp.tile([P, H], f32, tag="e1")
                nc.vector.scalar_tensor_tensor(
                    e1[:],
                    gt[:, 0, :],
                    dinv2s[:, m : m + 1],
                    b1dt[:],
                    mybir.AluOpType.mult,
                    mybir.AluOpType.add,
                )
                h2 = hp.tile([P, H], bf16, tag="h2")
                nc.scalar.activation(h2[:], e1[:], mybir.ActivationFunctionType.Relu)
                nc.sync.dma_start(ag2_in[m * P : (m + 1) * P, :], h2[:])
            else:
                h2 = hp.tile([P, H], bf16, tag="h2")
                nc.vector.tensor_copy(h2[:], gt[:, 0, :])
                nc.sync.dma_start(ag2_in[m * P : (m + 1) * P, :], h2[:])
            off += 8 * D

        if phase_limit >= 4:
            nc.gpsimd.collective_compute(
                "AllGather",
                mybir.AluOpType.bypass,
                replica_groups=[list(range(NCORES))],
                ins=[ag2_in[:].opt()],
                outs=[ag2_out[:].opt()],
            )

        # ---- layer-2 aggregation + GEMM by W2 ----
        off = 0
        for m in range(M if phase_limit >= 5 else 0):
            D = Dlist[m]
            gt = gp.tile([P, D, H], bf16, tag="gt")
            for j0 in range(0, D, 16):
                j1 = min(j0 + 16, D)
                nc.gpsimd.dma_gather(
                    gt[:, j0:j1, :],
                    ag2_out[:],
                    idxsb[:, off + 8 * j0 : off + 8 * j1],
                    P * (j1 - j0),
                    P * (j1 - j0),
                    H,
                    single_packet=False,
                    queue_num=(m * 4 + j0 // 16) % 4,
                )
            _tree_reduce(nc, gt, D)
            s2d = hp.tile([P, H], bf16, tag="s2d")
            nc.vector.tensor_scalar_mul(s2d[:], gt[:, 0, :], dinvs[:, m : m + 1])
            sts = []
            for kk in range(2):
                pt = ppt.tile([P, P], bf16, tag="pt")
                nc.tensor.transpose(pt[:], s2d[:, kk * P : (kk + 1) * P], identsb[:])
                st = sp.tile([P, P], bf16, tag="st")
                nc.scalar.copy(st[:], pt[:])
                sts.append(st)
            ps2 = pp2.tile([P, C], f32, tag="ps2")
            for kk in range(2):
                nc.tensor.matmul(
                    ps2[:], sts[kk][:], w2sb[kk][:], start=(kk == 0), stop=(kk == 1)
                )
            nc.vector.tensor_add(outsb[:, m * C : (m + 1) * C], ps2[:], b2bc[:])
            off += 8 * D

        nc.sync.dma_start(out_d[:], outsb[:])

    nc.compile()
    return nc


def kernel(x, edge_index, W1, b1, W2, b2):
    global last_result
    x = np.asarray(x, dtype=np.float32)
    edge_index = np.asarray(edge_index)
    W1 = np.asarray(W1, dtype=np.float32)
    b1 = np.asarray(b1, dtype=np.float32)
    W2 = np.asarray(W2, dtype=np.float32)
    b2 = np.asarray(b2, dtype=np.float32)

    n = x.shape[0]
    src = edge_index[0].astype(np.int64)
    dst = edge_index[1].astype(np.int64)

    # ---- normalization ----
    deg_in = np.bincount(dst, minlength=n).astype(np.int64)
    degv = deg_in.astype(np.float64) + 1.0
    dinv = (1.0 / np.sqrt(degv)).astype(np.float32)

    # ---- degree-sorted slot assignment ----
    order = np.argsort(-degv, kind="stable")          # rank -> node
    ranks = np.arange(NT, dtype=np.int64)
    g = ranks // P
    slot_of_rank = (g % NCORES) * NS + (g // NCORES) * P + (ranks % P)
    node_of_slot = np.full(NT, -1, dtype=np.int64)
    node_of_slot[slot_of_rank[:n]] = order
    slot_of_node = np.empty(n, dtype=np.int64)
    slot_of_node[order] = slot_of_rank[:n]

    dslot = slot_of_node[dst]
    sslot = slot_of_node[src]

    counts = np.bincount(dslot, minlength=NT).astype(np.int64)
    cnt1 = counts + 1                                  # + self edge
    mpos_of_slot = (np.arange(NT) % NS) // P
    Dm = np.zeros(M, dtype=np.int64)
    np.maximum.at(Dm, mpos_of_slot, cnt1)
    Dlist = tuple(int(v) for v in Dm)
    Dmax = int(Dm.max())

    PAD_SLOT = NT - 1 if n < NT else NT - 1            # guaranteed dummy (zero row)
    A = np.full((NT, Dmax), PAD_SLOT, dtype=np.int64)
    A[:, 0] = np.arange(NT)                            # self edge
    eorder = np.argsort(dslot, kind="stable")
    ds = dslot[eorder]
    ss = sslot[eorder]
    starts = np.zeros(NT + 1, dtype=np.int64)
    np.cumsum(counts, out=starts[1:])
    pos = np.arange(E, dtype=np.int64) - starts[ds]
    A[ds, pos + 1] = ss

    # ---- per-core inputs ----
    x_bf = x.astype(_BF16)
    w1_bf = W1.astype(_BF16)
    w2_bf = W2.astype(_BF16)
    b1bc = np.broadcast_to(b1, (P, H)).astype(np.float32).copy()
    b2bc = np.broadcast_to(b2, (P, C)).astype(np.float32).copy()
    ident = np.eye(P, dtype=np.float32).astype(_BF16)

    dinv_slots = np.zeros(NT, dtype=np.float32)
    real = node_of_slot >= 0
    dinv_slots[real] = dinv[node_of_slot[real]]

    in_maps = []
    for c in range(NCORES):
        slots = np.arange(c * NS, (c + 1) * NS)
        nos = node_of_slot[slots]
        xs = np.zeros((NS, F_IN), dtype=_BF16)
        r = nos >= 0
        xs[r] = x_bf[nos[r]]
        dv = dinv_slots[slots].reshape(M, P).T.copy()   # [128, M]
        blocks = []
        for m in range(M):
            blk = A[slots[m * P : (m + 1) * P], : Dlist[m]]   # [128, D]
            flat = blk.T.reshape(-1)                          # i = j*128 + p
            blocks.append(flat.reshape(-1, 16).T)             # [16, 8*D]
        idx16 = np.concatenate(blocks, axis=1)
        idx128 = np.tile(idx16, (8, 1)).astype(np.int16)
        in_maps.append(
            {
                "xT": np.ascontiguousarray(xs.T),
                "w1": w1_bf,
                "w2": w2_bf,
                "b1bc": b1bc,
                "b2bc": b2bc,
                "dinv": dv,
                "dinv2": dv * dv,
                "idx": idx128,
                "ident": ident,
            }
        )

    # ---- build + run ----
    from concourse.bass_utils import run_bass_kernel_spmd

    trace = bool(int(os.environ.get("BASS_GCN_TRACE", "0")))
    if trace:
        _install_trace_shim()

    key = Dlist
    if key not in _prog_cache:
        _prog_cache[key] = _build_program(Dlist)
    nc = _prog_cache[key]

    res = run_bass_kernel_spmd(nc, in_maps, list(range(NCORES)), trace=trace)
    last_result = res

    # ---- gather + unpermute ----
    out_full = np.empty((n, C), dtype=np.float32)
    for c in range(NCORES):
        oc = res.results[c]["out"].reshape(P, M, C).transpose(1, 0, 2).reshape(NS, C)
        slots = np.arange(c * NS, (c + 1) * NS)
        nos = node_of_slot[slots]
        r = nos >= 0
        out_full[nos[r]] = oc[r]

    return (out_full / np.float32(TEMPERATURE)).astype(np.float32)


# revision 13
# speedup vs baseline: 1.7826x; 1.1144x over previous
"""GCN (2-layer, symmetric-normalized, self-loops) on 8 Trainium2 NeuronCores.

Strategy (graph/data parallel):
  - Nodes are sorted by degree, grouped into 128-node chunks, and chunks are
    dealt round-robin to the 8 cores (load balance + tight ELL padding).
  - Normalization is factored: out_i = dinv_i * sum_{j in N(i)+self} (dinv_j h_j)
    so each layer is: GEMM -> row-scale by dinv -> AllGather the bf16 table ->
    per-dst-chunk padded gather (dma_gather, 512B rows) -> tree-reduce (DVE) ->
    epilogue.  Layer 2 ends with a small GEMM by W2 (PE transpose first).
  - All per-edge work is on-device; the host only builds int16 index tables.
"""

import os
import numpy as np
import ml_dtypes

N, F_IN, H, C, E = 20000, 512, 256, 40, 640000
TEMPERATURE = 1.0
NCORES = 8
P = 128
M = 20                     # node chunks per core
NS = M * P                 # node slots per core (2560)
NT = NCORES * NS           # total slots (20480)

_BF16 = ml_dtypes.bfloat16
_prog_cache = {}
last_result = None         # BassKernelResults of the most recent run (for test.py)


def _install_trace_shim():
    """Provide antenv.axon_hooks (NTFF profiling) missing from this image."""
    import contextlib, ctypes, sys, types
    import antenv

    if "antenv.axon_hooks" in sys.modules:
        return

    so_path = "/opt/axon/libaxon_pjrt.so"

    def _build_hook():
        try:
            lib = ctypes.CDLL(so_path)
        except OSError:
            return None
        if not hasattr(lib, "axon_start_nrt_profile"):
            return None
        lib.axon_start_nrt_profile.argtypes = [
            ctypes.POINTER(ctypes.c_int64),
            ctypes.c_size_t,
        ]
        lib.axon_start_nrt_profile.restype = ctypes.c_int64
        lib.axon_stop_nrt_profile.argtypes = [ctypes.c_char_p]
        lib.axon_stop_nrt_profile.restype = ctypes.c_int64

        @contextlib.contextmanager
        def _hook(output_dir, device_ids):
            import jax

            jax.devices()
            if device_ids:
                ids = (ctypes.c_int64 * len(device_ids))(*device_ids)
                rc = lib.axon_start_nrt_profile(ids, len(device_ids))
            else:
                rc = lib.axon_start_nrt_profile(None, 0)
            if rc != 0:
                raise RuntimeError(f"axon_start_nrt_profile rc={rc}")
            try:
                yield
            finally:
                lib.axon_stop_nrt_profile(output_dir.encode())

        return _hook

    mod = types.ModuleType("antenv.axon_hooks")
    _state = {"hook": None, "built": False}

    def get_axon_ntff_profile_hook():
        if not _state["built"]:
            _state["hook"] = _build_hook()
            _state["built"] = True
        return _state["hook"]

    def set_axon_ntff_profile_hook(h):
        _state["hook"] = h
        _state["built"] = True

    mod.get_axon_ntff_profile_hook = get_axon_ntff_profile_hook
    mod.set_axon_ntff_profile_hook = set_axon_ntff_profile_hook
    antenv.axon_hooks = mod
    sys.modules["antenv.axon_hooks"] = mod

    # hlo_convert binary is absent in this image; disable HLO annotation.
    import gauge.trn_perfetto as tp

    if not getattr(tp.main, "_no_hlo_patch", False):
        orig = tp.main

        def main(*a, **k):
            k["annotate_hlo"] = False
            return orig(*a, **k)

        main._no_hlo_patch = True
        tp.main = main


def _tree_reduce(nc, gt, D):
    """In-place sum of gt[:, 0:D, :] into gt[:, 0, :] (bf16 pairwise tree)."""
    n = D
    while n > 1:
        h = n // 2
        nc.vector.tensor_add(gt[:, 0:h, :], gt[:, 0:h, :], gt[:, h : 2 * h, :])
        if n % 2 == 1:
            nc.vector.tensor_add(gt[:, 0, :], gt[:, 0, :], gt[:, n - 1, :])
        n = h


def _build_program(Dlist):
    from contextlib import ExitStack

    import concourse.bacc as bacc
    import concourse.tile as tile
    import concourse.mybir as mybir
    from concourse.library_config import mlp

    bf16 = mybir.dt.bfloat16
    f32 = mybir.dt.float32
    i16 = mybir.dt.int16
    SIDX = 8 * sum(Dlist)

    phase_limit = int(os.environ.get("GCN_PHASE_LIMIT", "99"))
    nc = bacc.Bacc("TRN2", num_devices=NCORES, target_bir_lowering=True, num_swdge_queues=int(os.environ.get("GCN_NQ", "4")))

    xT_d = nc.dram_tensor("xT", [F_IN, NS], bf16, kind="ExternalInput")
    w1_d = nc.dram_tensor("w1", [F_IN, H], bf16, kind="ExternalInput")
    w2_d = nc.dram_tensor("w2", [H, C], bf16, kind="ExternalInput")
    b1_d = nc.dram_tensor("b1bc", [P, H], f32, kind="ExternalInput")
    b2_d = nc.dram_tensor("b2bc", [P, C], f32, kind="ExternalInput")
    dinv_d = nc.dram_tensor("dinv", [P, M], f32, kind="ExternalInput")
    dinv2_d = nc.dram_tensor("dinv2", [P, M], f32, kind="ExternalInput")
    idx_d = nc.dram_tensor("idx", [P, SIDX], i16, kind="ExternalInput")
    id_d = nc.dram_tensor("ident", [P, P], bf16, kind="ExternalInput")
    out_d = nc.dram_tensor("out", [P, M * C], f32, kind="ExternalOutput")

    with tile.TileContext(nc) as tc, ExitStack() as ctx:
        dp = ctx.enter_context(tc.tile_pool(name="dram", bufs=1, space="DRAM"))
        cp = ctx.enter_context(tc.tile_pool(name="const", bufs=1))
        pp1 = ctx.enter_context(tc.tile_pool(name="ps1", bufs=2, space="PSUM"))
        ppt = ctx.enter_context(tc.tile_pool(name="pt", bufs=3, space="PSUM"))
        pp2 = ctx.enter_context(tc.tile_pool(name="ps2", bufs=2, space="PSUM"))
        hp = ctx.enter_context(tc.tile_pool(name="hp", bufs=3))
        gp = ctx.enter_context(tc.tile_pool(name="gp", bufs=3))
        sp = ctx.enter_context(tc.tile_pool(name="sp", bufs=3))

        nc.gpsimd.load_library(mlp)

        ag1_in = dp.tile([NS, H], bf16)
        ag1_out = dp.tile([NT, H], bf16, addr_space="Shared")
        ag2_in = dp.tile([NS, P], bf16)
        ag2_out = dp.tile([NT, P], bf16, addr_space="Shared")

        w1sb = []
        for k in range(4):
            t = cp.tile([P, H], bf16, name=f"w1_{k}")
            nc.sync.dma_start(t[:], w1_d[k * P : (k + 1) * P, :])
            w1sb.append(t)
        w2sb = []
        for k in range(2):
            t = cp.tile([P, C], bf16, name=f"w2_{k}")
            nc.sync.dma_start(t[:], w2_d[k * P : (k + 1) * P, :])
            w2sb.append(t)
        xts = []
        for k in range(4):
            t = cp.tile([P, NS], bf16, name=f"xt_{k}")
            nc.sync.dma_start(t[:], xT_d[k * P : (k + 1) * P, :])
            xts.append(t)
        b1bc = cp.tile([P, H], f32, name="b1bc")
        nc.sync.dma_start(b1bc[:], b1_d[:])
        b2bc = cp.tile([P, C], f32, name="b2bc")
        nc.sync.dma_start(b2bc[:], b2_d[:])
        dinvs = cp.tile([P, M], f32, name="dinvs")
        nc.sync.dma_start(dinvs[:], dinv_d[:])
        dinv2s = cp.tile([P, M], f32, name="dinv2s")
        nc.sync.dma_start(dinv2s[:], dinv2_d[:])
        idxsb = cp.tile([P, SIDX], i16, name="idxsb")
        nc.sync.dma_start(idxsb[:], idx_d[:])
        identsb = cp.tile([P, P], bf16, name="identsb")
        nc.sync.dma_start(identsb[:], id_d[:])
        outsb = cp.tile([P, M * C], f32, name="outsb")
        if phase_limit < 5:
            nc.vector.memset(outsb[:], 0.0)

        # ---- layer-1 GEMM: h1 = dinv * (x @ W1), per 128-node chunk ----
        for m in range(M):
            ps = pp1.tile([P, H], f32, tag="ps1")
            for k in range(4):
                nc.tensor.matmul(
                    ps[:],
                    xts[k][:, m * P : (m + 1) * P],
                    w1sb[k][:],
                    start=(k == 0),
                    stop=(k == 3),
                )
            h1 = hp.tile([P, H], bf16, tag="h1")
            nc.scalar.mul(h1[:], ps[:], dinvs[:, m : m + 1])
            nc.sync.dma_start(ag1_in[m * P : (m + 1) * P, :], h1[:])

        if phase_limit >= 2:
            nc.gpsimd.collective_compute(
                "AllGather",
                mybir.AluOpType.bypass,
                replica_groups=[list(range(NCORES))],
                ins=[ag1_in[:].opt()],
                outs=[ag1_out[:].opt()],
            )

        # ---- layer-1 aggregation + relu -> h2 table ----
        ph3_mode = int(os.environ.get("GCN_PH3_MODE", "3"))
        off = 0
        for m in range(M if phase_limit >= 3 else 0):
            D = Dlist[m]
            gt = gp.tile([P, D, H], bf16, tag="gt")
            for j0 in range(0, D, 16):
                j1 = min(j0 + 16, D)
                nc.gpsimd.dma_gather(
                    gt[:, j0:j1, :],
                    ag1_out[:],
                    idxsb[:, off + 8 * j0 : off + 8 * j1],
                    P * (j1 - j0),
                    P * (j1 - j0),
                    H,
                    single_packet=False,
                    queue_num=(m * 4 + j0 // 16) % int(os.environ.get("GCN_NQ", "4")),
                )
            if ph3_mode >= 2:
                _tree_reduce(nc, gt, D)
            if ph3_mode >= 3:
                b1dt = sp.tile([P, H], f32, tag="b1d")
                nc.vector.tensor_scalar_mul(b1dt[:], b1bc[:], dinvs[:, m : m + 1])
                e1 = sp.tile([P, H], f32, tag="e1")
                nc.vector.scalar_tensor_tensor(
                    e1[:],
                    gt[:, 0, :],
                    dinv2s[:, m : m + 1],
                    b1dt[:],
                    mybir.AluOpType.mult,
                    mybir.AluOpType.add,
                )
                h2 = hp.tile([P, H], bf16, tag="h2")
                nc.scalar.activation(h2[:], e1[:], mybir.ActivationFunctionType.Relu)
                sts = []
                for kk in range(2):
                    pt = ppt.tile([P, P], bf16, tag="pt")
                    nc.tensor.transpose(
                        pt[:], h2[:, kk * P : (kk + 1) * P], identsb[:]
                    )
                    st = sp.tile([P, P], bf16, tag="st")
                    nc.scalar.copy(st[:], pt[:])
                    sts.append(st)
                ps2 = pp2.tile([P, C], f32, tag="ps2")
                for kk in range(2):
                    nc.tensor.matmul(
                        ps2[:], sts[kk][:], w2sb[kk][:], start=(kk == 0), stop=(kk == 1)
                    )
                hw2 = hp.tile([P, P], bf16, tag="hw2")
                nc.vector.memset(hw2[:], 0.0)
                nc.scalar.copy(hw2[:, :C], ps2[:])
                nc.sync.dma_start(ag2_in[m * P : (m + 1) * P, :], hw2[:])
            else:
                h2 = hp.tile([P, H], bf16, tag="h2")
                nc.vector.tensor_copy(h2[:], gt[:, 0, :])
                nc.sync.dma_start(ag2_in[m * P : (m + 1) * P, :], h2[:])
            off += 8 * D

        if phase_limit >= 4:
            nc.gpsimd.collective_compute(
                "AllGather",
                mybir.AluOpType.bypass,
                replica_groups=[list(range(NCORES))],
                ins=[ag2_in[:].opt()],
                outs=[ag2_out[:].opt()],
            )

        # ---- layer-2 aggregation + GEMM by W2 ----
        off = 0
        for m in range(M if phase_limit >= 5 else 0):
            D = Dlist[m]
            gt = gp.tile([P, D, P], bf16, tag="gt2")
            for j0 in range(0, D, 16):
                j1 = min(j0 + 16, D)
                nc.gpsimd.dma_gather(
                    gt[:, j0:j1, :],
                    ag2_out[:],
                    idxsb[:, off + 8 * j0 : off + 8 * j1],
                    P * (j1 - j0),
                    P * (j1 - j0),
                    P,
                    single_packet=False,
                    queue_num=(m * 4 + j0 // 16) % int(os.environ.get("GCN_NQ", "4")),
                )
            _tree_reduce(nc, gt, D)
            nc.vector.scalar_tensor_tensor(
                outsb[:, m * C : (m + 1) * C],
                gt[:, 0, :C],
                dinvs[:, m : m + 1],
                b2bc[:],
                mybir.AluOpType.mult,
                mybir.AluOpType.add,
            )
            off += 8 * D

        nc.sync.dma_start(out_d[:], outsb[:])

    nc.compile()
    return nc


def kernel(x, edge_index, W1, b1, W2, b2):
    global last_result
    x = np.asarray(x, dtype=np.float32)
    edge_index = np.asarray(edge_index)
    W1 = np.asarray(W1, dtype=np.float32)
    b1 = np.asarray(b1, dtype=np.float32)
    W2 = np.asarray(W2, dtype=np.float32)
    b2 = np.asarray(b2, dtype=np.float32)

    n = x.shape[0]
    src = edge_index[0].astype(np.int64)
    dst = edge_index[1].astype(np.int64)

    # ---- normalization ----
    deg_in = np.bincount(dst, minlength=n).astype(np.int64)
    degv = deg_in.astype(np.float64) + 1.0
    dinv = (1.0 / np.sqrt(degv)).astype(np.float32)

    # ---- degree-sorted slot assignment ----
    order = np.argsort(-degv, kind="stable")          # rank -> node
    ranks = np.arange(NT, dtype=np.int64)
    g = ranks // P
    slot_of_rank = (g % NCORES) * NS + (g // NCORES) * P + (ranks % P)
    # table row for a slot under half-split AllGather layout:
    # row = half*NT/2 + core*(NS/2) + (pos % (NS/2))
    all_slots = np.arange(NT, dtype=np.int64)
    core_of_slot = all_slots // NS
    pos_of_slot = all_slots % NS
    row_of_slot = all_slots
    node_of_slot = np.full(NT, -1, dtype=np.int64)
    node_of_slot[slot_of_rank[:n]] = order
    slot_of_node = np.empty(n, dtype=np.int64)
    slot_of_node[order] = slot_of_rank[:n]

    dslot = slot_of_node[dst]
    sslot = slot_of_node[src]

    counts = np.bincount(dslot, minlength=NT).astype(np.int64)
    cnt1 = counts + 1                                  # + self edge
    mpos_of_slot = (np.arange(NT) % NS) // P
    Dm = np.zeros(M, dtype=np.int64)
    np.maximum.at(Dm, mpos_of_slot, cnt1)
    Dlist = tuple(int(v) for v in Dm)
    Dmax = int(Dm.max())

    PAD_ROW = row_of_slot[NT - 1]                      # guaranteed dummy (zero row)
    A = np.full((NT, Dmax), PAD_ROW, dtype=np.int64)
    A[:, 0] = row_of_slot                               # self edge
    ss_rows = row_of_slot[sslot]
    eorder = np.argsort(dslot, kind="stable")
    ds = dslot[eorder]
    ss = ss_rows[eorder]
    starts = np.zeros(NT + 1, dtype=np.int64)
    np.cumsum(counts, out=starts[1:])
    pos = np.arange(E, dtype=np.int64) - starts[ds]
    A[ds, pos + 1] = ss

    # ---- per-core inputs ----
    x_bf = x.astype(_BF16)
    w1_bf = W1.astype(_BF16)
    w2_bf = W2.astype(_BF16)
    b1bc = np.broadcast_to(b1, (P, H)).astype(np.float32).copy()
    b2bc = np.broadcast_to(b2, (P, C)).astype(np.float32).copy()
    ident = np.eye(P, dtype=np.float32).astype(_BF16)

    dinv_slots = np.zeros(NT, dtype=np.float32)
    real = node_of_slot >= 0
    dinv_slots[real] = dinv[node_of_slot[real]]

    in_maps = []
    for c in range(NCORES):
        slots = np.arange(c * NS, (c + 1) * NS)
        nos = node_of_slot[slots]
        xs = np.zeros((NS, F_IN), dtype=_BF16)
        r = nos >= 0
        xs[r] = x_bf[nos[r]]
        dv = dinv_slots[slots].reshape(M, P).T.copy()   # [128, M]
        blocks = []
        for m in range(M):
            blk = A[slots[m * P : (m + 1) * P], : Dlist[m]]   # [128, D]
            flat = blk.T.reshape(-1)                          # i = j*128 + p
            blocks.append(flat.reshape(-1, 16).T)             # [16, 8*D]
        idx16 = np.concatenate(blocks, axis=1)
        idx128 = np.tile(idx16, (8, 1)).astype(np.int16)
        in_maps.append(
            {
                "xT": np.ascontiguousarray(xs.T),
                "w1": w1_bf,
                "w2": w2_bf,
                "b1bc": b1bc,
                "b2bc": b2bc,
                "dinv": dv,
                "dinv2": dv * dv,
                "idx": idx128,
                "ident": ident,
            }
        )

    # ---- build + run ----
    from concourse.bass_utils import run_bass_kernel_spmd

    trace = bool(int(os.environ.get("BASS_GCN_TRACE", "0")))
    if trace:
        _install_trace_shim()

    key = Dlist
    if key not in _prog_cache:
        _prog_cache[key] = _build_program(Dlist)
    nc = _prog_cache[key]

    res = run_bass_kernel_spmd(nc, in_maps, list(range(NCORES)), trace=trace)
    last_result = res

    # ---- gather + unpermute ----
    out_full = np.empty((n, C), dtype=np.float32)
    for c in range(NCORES):
        oc = res.results[c]["out"].reshape(P, M, C).transpose(1, 0, 2).reshape(NS, C)
        slots = np.arange(c * NS, (c + 1) * NS)
        nos = node_of_slot[slots]
        r = nos >= 0
        out_full[nos[r]] = oc[r]

    return (out_full / np.float32(TEMPERATURE)).astype(np.float32)
